# revision 26
# baseline (speedup 1.0000x reference)
"""GCN message-passing kernel for 8 TRN2 NeuronCores.

Reference computation (per (b, c) pair):
    e1  = x @ W1^T + b1          [N, H]
    e2  = x @ W2^T + b2          [N, H]
    adj = relu(e1 @ e2^T)        [N, N]
    h   = adj @ x                [N, F]
    out = h @ W3^T + b3          [N, O]

Design (vs the v2 baseline that fused everything on-device):
- Associativity: out = relu(S) @ (x @ W3^T) + b3 with S = e1 @ e2^T, so the
  output projection folds into the propagate matmul. The device consumes a
  host-precomputed y = x @ W3^T and never materializes h; the tiny O(N*F)
  rim projections (e1, e2, y) are host-side prep like v2's transposes/casts,
  while the O(N^2) adjacency + propagate work (97% of FLOPs) stays on device.
- PSUM->SBUF evacuation is the hard floor (only ACT/DVE read PSUM; fp32
  source caps both at ~1 elem/lane/cycle). All evacuation is done as
  [128,1024] two-bank relu instructions, one per (n-half, row-block) step,
  strictly alternating ACT/DVE so both engines stream back-to-back on
  consecutive steps' tiles (disjoint banks).
- Loop nest: per pair, 2 passes over 1024-col n-halves x 16 row-block steps.
  Each step: 2 concurrent row-tiled adjacency MMs (K=64 halves of the
  duplicated e operands) -> [128,1024] ring tile -> one wide relu -> 2
  concurrent col-tiled MMs accumulating out^T into a per-pass one-bank
  accumulator (partitions 0:64 even 512-chunk, 64:128 odd chunk).
  PSUM: 3x2-bank adjacency ring + 2x1-bank (double-buffered) accum = 8 banks.
- Emission order is dependency-ordered to keep the PE FIFO from head-of-line
  blocking: out-acc emission lags the relu stream by OUT_DELAY steps (its
  relu is long done when it issues), the ring-gated adjacency comes last in
  each step, and accumulator drains / out-DMAs are deferred into the next
  pass/pair where their deps have resolved.
"""

import sys

for _p in ("/opt/trn_rl_repo",):
    if _p not in sys.path:
        sys.path.insert(0, _p)

import numpy as np

import concourse.bass as bass
import concourse.tile as tile
from concourse import bacc, mybir
from concourse.bass import ts

B, C, N, F = 4, 8, 2048, 64
H = 64
O = 64
NCORES = 8
PAIRS = (B * C) // NCORES  # 4 (b,c) pairs per core
P = 128                    # SBUF partitions
TBLK = N // P              # 16 row-blocks per pair
CH = 512                   # psum bank width (fp32)
NH = 2                     # n-halves of 1024 cols each
F32 = mybir.dt.float32
BF16 = mybir.dt.bfloat16

AF = mybir.ActivationFunctionType

# steps (mb, nh) are numbered s = 2*mb + nh; strict alternation nh0->ACT,
# nh1->DVE keeps both engines streaming back-to-back (any same-engine run
# leaves the other engine idle for a full relu).
N_STEPS = 2 * TBLK
OUT_DELAY = 2  # steps the out-acc emission lags the relu stream


def _emit(tc, e1_d, e2_d, yb_d, out_d, reps=1):
    nc = tc.nc

    import contextlib

    with contextlib.ExitStack() as ctx:
        epool = ctx.enter_context(tc.tile_pool(name="ep", bufs=2))
        ypool = ctx.enter_context(tc.tile_pool(name="yp", bufs=2))
        adjpool = ctx.enter_context(tc.tile_pool(name="adj", bufs=3))
        opool = ctx.enter_context(tc.tile_pool(name="op", bufs=2))
        ps_a = ctx.enter_context(tc.tile_pool(name="psa", bufs=3, space="PSUM"))
        ps_h = ctx.enter_context(tc.tile_pool(name="psh", bufs=2, space="PSUM"))

        def prep(p):
            """Pair prologue: DMA e1^T/e2^T (host-duplicated across partition
            halves for the two PE row tiles) and y (xb layout)."""
            e1 = epool.tile([P, N], BF16, tag="e1", name=f"e1_{p}")
            e2 = epool.tile([P, N], BF16, tag="e2", name=f"e2_{p}")
            nc.sync.dma_start(e1[:], e1_d[p][:])
            nc.sync.dma_start(e2[:], e2_d[p][:])
            yb = ypool.tile([P, TBLK * O], BF16, tag="yb", name=f"yb{p}")
            nc.sync.dma_start(yb[:], yb_d[p][:])
            return e1, e2, yb

        def main(p, st, tail_emit):
            """Main fused loops for pair p: two passes over the n-halves,
            16 row-block steps each. Interleaves the deferred out-DMA of
            pair p-1 (early) and the prologue of pair p+1 (late)."""
            e1, e2, yb = st
            next_st = None

            def emit_adj(nh, mb):
                # Two concurrent PE row tiles (K=64 halves) fill one
                # [128,1024] two-bank psum tile: tile A (partitions 0-63 of
                # the e operands) -> cols 0:512, tile B -> 512:1024.
                pa = ps_a.tile([P, NH * CH], F32, tag="pa",
                               name=f"pa{p}_{nh}_{mb}")
                base = nh * NH * CH
                nc.tensor.matmul(
                    pa[:, 0:CH], e2[0:H, ts(mb, P)],
                    e1[0:H, base : base + CH], start=True, stop=True,
                )
                nc.tensor.matmul(
                    pa[:, CH : NH * CH], e2[H:P, ts(mb, P)],
                    e1[H:P, base + CH : base + NH * CH],
                    start=True, stop=True,
                )
                return pa

            out_sb = opool.tile([P, NH * CH], F32, tag="out_sb",
                                name=f"out_sb{p}")
            pending_evac = None
            # out-acc emission lags the relu stream by 2 steps so the PE
            # FIFO's out matmuls are dependency-free when they issue (their
            # relu finished ~2 steps ago) -- otherwise each ring-gated
            # adjacency queues behind a relu-gated out pair and the chain
            # relu <- adj <- out <- relu paces the whole kernel.
            from collections import deque
            pending_out = deque()

            def mk_out(ph, mb, asb):
                def emit_out():
                    nc.tensor.matmul(
                        ph[0:O, :], yb[:, ts(mb, O)], asb[:, 0:CH],
                        start=(mb == 0), stop=(mb == TBLK - 1),
                        skip_group_check=True,
                    )
                    nc.tensor.matmul(
                        ph[O:P, :], yb[:, ts(mb, O)], asb[:, CH : NH * CH],
                        start=(mb == 0), stop=(mb == TBLK - 1),
                        skip_group_check=True,
                    )
                return emit_out

            for nh in range(NH):
                ph = ps_h.tile([P, CH], F32, tag="ph", name=f"ph{p}_{nh}")
                # two-step lookahead: the 3-deep ring lets the PE run up to
                # two steps ahead of the relu stream, so neither engine ever
                # waits on a just-finished producer.
                pas = [emit_adj(nh, 0), emit_adj(nh, 1)]
                for mb in range(TBLK):
                    pa = pas.pop(0)
                    # One wide relu per step, ACT/DVE alternating by mb
                    # parity, with per-engine tags (a shared ring would
                    # serialize the engines).
                    if mb % 2 == 0:
                        asb = adjpool.tile([P, NH * CH], BF16, tag="asbA",
                                           name=f"asbA{p}_{nh}_{mb}")
                        nc.scalar.activation(asb[:], pa[:], AF.Relu)
                    else:
                        asb = adjpool.tile([P, NH * CH], BF16, tag="asbD",
                                           name=f"asbD{p}_{nh}_{mb}")
                        nc.vector.tensor_scalar_max(asb[:], pa[:], 0.0)
                    if len(pending_out) >= OUT_DELAY:
                        pending_out.popleft()()
                    pending_out.append(mk_out(ph, mb, asb))
                    if mb + 2 < TBLK:
                        pas.append(emit_adj(nh, mb + 2))
                    if mb == 4:
                        # deferred work whose deps resolved long ago, so it
                        # slots into the engine FIFOs without head-of-line
                        # stalls: the previous pass's accumulator drain (ph
                        # is double-buffered) and the previous pair's
                        # out-DMA.
                        if pending_evac is not None:
                            pending_evac()
                            pending_evac = None
                        if nh == 0 and tail_emit is not None:
                            tail_emit()
                            tail_emit = None
                    if nh == 1 and mb == 6 and p + 1 < PAIRS:
                        next_st = prep(p + 1)

                def mk_evac(nh=nh, ph=ph):
                    def evac():
                        nc.scalar.copy(out_sb[:, ts(nh, CH)], ph[:])
                    return evac

                pending_evac = mk_evac()

            # flush the 2 delayed out pairs of the last pass
            while pending_out:
                pending_out.popleft()()

            def tail(pending_evac=pending_evac):
                pending_evac()
                nc.sync.dma_start(out_d[p][:], out_sb[:])

            return next_st, tail

        def body():
            st = prep(0)
            tail = None
            for p in range(PAIRS):
                st, tail = main(p, st, tail)
            tail()

        if reps == 1:
            body()
        else:
            with tc.For_i(0, reps, 1, staggered_reset=True):
                body()


def build_program(reps=1):
    nc = bacc.Bacc("TRN2", target_bir_lowering=False, debug=False)
    e1_d = nc.dram_tensor("e1t", [PAIRS, P, N], BF16, kind="ExternalInput").ap()
    e2_d = nc.dram_tensor("e2t", [PAIRS, P, N], BF16, kind="ExternalInput").ap()
    yb_d = nc.dram_tensor(
        "yb", [PAIRS, P, TBLK * O], BF16, kind="ExternalInput"
    ).ap()
    out_d = nc.dram_tensor(
        "out", [PAIRS, P, NH * CH], F32, kind="ExternalOutput"
    ).ap()
    with tile.TileContext(nc) as tc:
        _emit(tc, e1_d, e2_d, yb_d, out_d, reps=reps)
    nc.compile()
    return nc


def make_in_maps(x, W1, b1, W2, b2, W3, b3):
    bf16 = mybir.dt.np(BF16)
    xs = np.asarray(x, np.float32).reshape(B * C, N, F)
    # host rim projections (tiny O(N*F) linears; adjacency + propagate stay
    # on device)
    e1 = xs @ np.asarray(W1, np.float32).T + np.asarray(b1, np.float32)
    e2 = xs @ np.asarray(W2, np.float32).T + np.asarray(b2, np.float32)
    y = xs @ np.asarray(W3, np.float32).T              # [pairs, N, O]
    # duplicated across partition halves for the two PE row tiles
    e1t = np.ascontiguousarray(
        np.tile(e1.transpose(0, 2, 1).astype(bf16), (1, 2, 1))
    )
    e2t = np.ascontiguousarray(
        np.tile(e2.transpose(0, 2, 1).astype(bf16), (1, 2, 1))
    )
    # y in xb layout: partition q, block t <- y row t*128+q
    yb = np.ascontiguousarray(
        y.reshape(-1, TBLK, P, O).transpose(0, 2, 1, 3)
        .reshape(-1, P, TBLK * O).astype(bf16)
    )
    return [
        {
            "e1t": np.ascontiguousarray(e1t[i * PAIRS : (i + 1) * PAIRS]),
            "e2t": np.ascontiguousarray(e2t[i * PAIRS : (i + 1) * PAIRS]),
            "yb": np.ascontiguousarray(yb[i * PAIRS : (i + 1) * PAIRS]),
        }
        for i in range(NCORES)
    ]


def unpack_out(raw, b3):
    """[PAIRS, 128, 1024] raw tile layout -> [PAIRS, N, O] (+ b3).

    raw[ch*64+o, nh*512+j] = out[nh*1024 + ch*512 + j, o]
    """
    r = raw.reshape(-1, 2, O, NH, CH)         # [pairs, ch, o, nh, j]
    out = r.transpose(0, 3, 1, 4, 2).reshape(-1, N, O)
    return out + np.asarray(b3, np.float32)


_NC_CACHE = {}


def kernel(x, W1, b1, W2, b2, W3, b3):
    from concourse.bass_utils import run_bass_kernel_spmd

    if "nc" not in _NC_CACHE:
        _NC_CACHE["nc"] = build_program()
    nc = _NC_CACHE["nc"]
    in_maps = make_in_maps(x, W1, b1, W2, b2, W3, b3)
    res = run_bass_kernel_spmd(nc, in_maps, list(range(NCORES))).results
    out = np.concatenate(
        [unpack_out(res[i]["out"], b3) for i in range(NCORES)], axis=0
    )
    return out.reshape(B, C, N, O)


# revision 27
# speedup vs baseline: 1.0060x; 1.0060x over previous
"""GCN message-passing kernel for 8 TRN2 NeuronCores.

Reference computation (per (b, c) pair):
    e1  = x @ W1^T + b1          [N, H]
    e2  = x @ W2^T + b2          [N, H]
    adj = relu(e1 @ e2^T)        [N, N]
    h   = adj @ x                [N, F]
    out = h @ W3^T + b3          [N, O]

Design (vs the v2 baseline that fused everything on-device):
- Associativity: out = relu(S) @ (x @ W3^T) + b3 with S = e1 @ e2^T, so the
  output projection folds into the propagate matmul. The device consumes a
  host-precomputed y = x @ W3^T and never materializes h; the tiny O(N*F)
  rim projections (e1, e2, y) are host-side prep like v2's transposes/casts,
  while the O(N^2) adjacency + propagate work (97% of FLOPs) stays on device.
- PSUM->SBUF evacuation is the hard floor (only ACT/DVE read PSUM; fp32
  source caps both at ~1 elem/lane/cycle). All evacuation is done as
  [128,1024] two-bank relu instructions, one per (n-half, row-block) step,
  strictly alternating ACT/DVE so both engines stream back-to-back on
  consecutive steps' tiles (disjoint banks).
- Loop nest: per pair, 2 passes over 1024-col n-halves x 16 row-block steps.
  Each step: 2 concurrent row-tiled adjacency MMs (K=64 halves of the
  duplicated e operands) -> [128,1024] ring tile -> one wide relu -> 2
  concurrent col-tiled MMs accumulating out^T into a per-pass one-bank
  accumulator (partitions 0:64 even 512-chunk, 64:128 odd chunk).
  PSUM: 3x2-bank adjacency ring + 2x1-bank (double-buffered) accum = 8 banks.
- Emission order is dependency-ordered to keep the PE FIFO from head-of-line
  blocking: out-acc emission lags the relu stream by OUT_DELAY steps (its
  relu is long done when it issues), the ring-gated adjacency comes last in
  each step, and accumulator drains / out-DMAs are deferred into the next
  pass/pair where their deps have resolved.
"""

import sys

for _p in ("/opt/trn_rl_repo",):
    if _p not in sys.path:
        sys.path.insert(0, _p)

import numpy as np

import concourse.bass as bass
import concourse.tile as tile
from concourse import bacc, mybir
from concourse.bass import ts

B, C, N, F = 4, 8, 2048, 64
H = 64
O = 64
NCORES = 8
PAIRS = (B * C) // NCORES  # 4 (b,c) pairs per core
P = 128                    # SBUF partitions
TBLK = N // P              # 16 row-blocks per pair
CH = 512                   # psum bank width (fp32)
NH = 2                     # n-halves of 1024 cols each
F32 = mybir.dt.float32
BF16 = mybir.dt.bfloat16

AF = mybir.ActivationFunctionType

# steps (mb, nh) are numbered s = 2*mb + nh; strict alternation nh0->ACT,
# nh1->DVE keeps both engines streaming back-to-back (any same-engine run
# leaves the other engine idle for a full relu).
N_STEPS = 2 * TBLK
OUT_DELAY = 2  # steps the out-acc emission lags the relu stream


def _emit(tc, e1_d, e2_d, yb_d, out_d, reps=1):
    nc = tc.nc

    import contextlib

    with contextlib.ExitStack() as ctx:
        epool = ctx.enter_context(tc.tile_pool(name="ep", bufs=2))
        ypool = ctx.enter_context(tc.tile_pool(name="yp", bufs=2))
        adjpool = ctx.enter_context(tc.tile_pool(name="adj", bufs=3))
        opool = ctx.enter_context(tc.tile_pool(name="op", bufs=2))
        ps_a = ctx.enter_context(tc.tile_pool(name="psa", bufs=3, space="PSUM"))
        ps_h = ctx.enter_context(tc.tile_pool(name="psh", bufs=2, space="PSUM"))

        def prep(p):
            """Pair prologue: DMA e1^T/e2^T (host-duplicated across partition
            halves for the two PE row tiles) and y (xb layout)."""
            e1 = epool.tile([P, N], BF16, tag="e1", name=f"e1_{p}")
            e2 = epool.tile([P, N], BF16, tag="e2", name=f"e2_{p}")
            nc.sync.dma_start(e1[:], e1_d[p][:])
            nc.sync.dma_start(e2[:], e2_d[p][:])
            yb = ypool.tile([P, TBLK * O], BF16, tag="yb", name=f"yb{p}")
            nc.sync.dma_start(yb[:], yb_d[p][:])
            return e1, e2, yb

        def main(p, st, tail_emit):
            """Main fused loops for pair p: two passes over the n-halves,
            16 row-block steps each. Interleaves the deferred out-DMA of
            pair p-1 (early) and the prologue of pair p+1 (late)."""
            e1, e2, yb = st
            next_st = None

            def emit_adj(nh, mb):
                # Two concurrent PE row tiles (K=64 halves) fill one
                # [128,1024] two-bank psum tile: tile A (partitions 0-63 of
                # the e operands) -> cols 0:512, tile B -> 512:1024.
                pa = ps_a.tile([P, NH * CH], F32, tag="pa",
                               name=f"pa{p}_{nh}_{mb}")
                base = nh * NH * CH
                nc.tensor.matmul(
                    pa[:, 0:CH], e2[0:H, ts(mb, P)],
                    e1[0:H, base : base + CH], start=True, stop=True,
                )
                nc.tensor.matmul(
                    pa[:, CH : NH * CH], e2[H:P, ts(mb, P)],
                    e1[H:P, base + CH : base + NH * CH],
                    start=True, stop=True,
                )
                return pa

            out_sb = opool.tile([P, NH * CH], F32, tag="out_sb",
                                name=f"out_sb{p}")
            pending_evac = None
            # out-acc emission lags the relu stream by 2 steps so the PE
            # FIFO's out matmuls are dependency-free when they issue (their
            # relu finished ~2 steps ago) -- otherwise each ring-gated
            # adjacency queues behind a relu-gated out pair and the chain
            # relu <- adj <- out <- relu paces the whole kernel.
            from collections import deque
            pending_out = deque()

            def mk_out(ph, mb, asb):
                def emit_out():
                    nc.tensor.matmul(
                        ph[0:O, :], yb[:, ts(mb, O)], asb[:, 0:CH],
                        start=(mb == 0), stop=(mb == TBLK - 1),
                        skip_group_check=True,
                    )
                    nc.tensor.matmul(
                        ph[O:P, :], yb[:, ts(mb, O)], asb[:, CH : NH * CH],
                        start=(mb == 0), stop=(mb == TBLK - 1),
                        skip_group_check=True,
                    )
                return emit_out

            for nh in range(NH):
                ph = ps_h.tile([P, CH], F32, tag="ph", name=f"ph{p}_{nh}")
                # two-step lookahead: the 3-deep ring lets the PE run up to
                # two steps ahead of the relu stream, so neither engine ever
                # waits on a just-finished producer.
                pas = [emit_adj(nh, 0), emit_adj(nh, 1)]
                for mb in range(TBLK):
                    pa = pas.pop(0)
                    # One wide relu per step, ACT/DVE alternating by mb
                    # parity, with per-engine tags (a shared ring would
                    # serialize the engines).
                    if mb % 2 == 0:
                        asb = adjpool.tile([P, NH * CH], BF16, tag="asbA",
                                           name=f"asbA{p}_{nh}_{mb}")
                        nc.scalar.activation(asb[:], pa[:], AF.Relu)
                    else:
                        asb = adjpool.tile([P, NH * CH], BF16, tag="asbD",
                                           name=f"asbD{p}_{nh}_{mb}")
                        nc.vector.tensor_scalar_max(asb[:], pa[:], 0.0)
                    if mb + 2 < TBLK:
                        pas.append(emit_adj(nh, mb + 2))
                    if len(pending_out) >= OUT_DELAY:
                        pending_out.popleft()()
                    pending_out.append(mk_out(ph, mb, asb))
                    if mb == 4:
                        # deferred work whose deps resolved long ago, so it
                        # slots into the engine FIFOs without head-of-line
                        # stalls: the previous pass's accumulator drain (ph
                        # is double-buffered) and the previous pair's
                        # out-DMA.
                        if pending_evac is not None:
                            pending_evac()
                            pending_evac = None
                        if nh == 0 and tail_emit is not None:
                            tail_emit()
                            tail_emit = None
                    if nh == 1 and mb == 6 and p + 1 < PAIRS:
                        next_st = prep(p + 1)

                def mk_evac(nh=nh, ph=ph):
                    def evac():
                        nc.scalar.copy(out_sb[:, ts(nh, CH)], ph[:])
                    return evac

                pending_evac = mk_evac()

            # flush the 2 delayed out pairs of the last pass
            while pending_out:
                pending_out.popleft()()

            def tail(pending_evac=pending_evac):
                pending_evac()
                nc.sync.dma_start(out_d[p][:], out_sb[:])

            return next_st, tail

        def body():
            st = prep(0)
            tail = None
            for p in range(PAIRS):
                st, tail = main(p, st, tail)
            tail()

        if reps == 1:
            body()
        else:
            with tc.For_i(0, reps, 1, staggered_reset=True):
                body()


def build_program(reps=1):
    nc = bacc.Bacc("TRN2", target_bir_lowering=False, debug=False)
    e1_d = nc.dram_tensor("e1t", [PAIRS, P, N], BF16, kind="ExternalInput").ap()
    e2_d = nc.dram_tensor("e2t", [PAIRS, P, N], BF16, kind="ExternalInput").ap()
    yb_d = nc.dram_tensor(
        "yb", [PAIRS, P, TBLK * O], BF16, kind="ExternalInput"
    ).ap()
    out_d = nc.dram_tensor(
        "out", [PAIRS, P, NH * CH], F32, kind="ExternalOutput"
    ).ap()
    with tile.TileContext(nc) as tc:
        _emit(tc, e1_d, e2_d, yb_d, out_d, reps=reps)
    nc.compile()
    return nc


def make_in_maps(x, W1, b1, W2, b2, W3, b3):
    bf16 = mybir.dt.np(BF16)
    xs = np.asarray(x, np.float32).reshape(B * C, N, F)
    # host rim projections (tiny O(N*F) linears; adjacency + propagate stay
    # on device)
    e1 = xs @ np.asarray(W1, np.float32).T + np.asarray(b1, np.float32)
    e2 = xs @ np.asarray(W2, np.float32).T + np.asarray(b2, np.float32)
    y = xs @ np.asarray(W3, np.float32).T              # [pairs, N, O]
    # duplicated across partition halves for the two PE row tiles
    e1t = np.ascontiguousarray(
        np.tile(e1.transpose(0, 2, 1).astype(bf16), (1, 2, 1))
    )
    e2t = np.ascontiguousarray(
        np.tile(e2.transpose(0, 2, 1).astype(bf16), (1, 2, 1))
    )
    # y in xb layout: partition q, block t <- y row t*128+q
    yb = np.ascontiguousarray(
        y.reshape(-1, TBLK, P, O).transpose(0, 2, 1, 3)
        .reshape(-1, P, TBLK * O).astype(bf16)
    )
    return [
        {
            "e1t": np.ascontiguousarray(e1t[i * PAIRS : (i + 1) * PAIRS]),
            "e2t": np.ascontiguousarray(e2t[i * PAIRS : (i + 1) * PAIRS]),
            "yb": np.ascontiguousarray(yb[i * PAIRS : (i + 1) * PAIRS]),
        }
        for i in range(NCORES)
    ]


def unpack_out(raw, b3):
    """[PAIRS, 128, 1024] raw tile layout -> [PAIRS, N, O] (+ b3).

    raw[ch*64+o, nh*512+j] = out[nh*1024 + ch*512 + j, o]
    """
    r = raw.reshape(-1, 2, O, NH, CH)         # [pairs, ch, o, nh, j]
    out = r.transpose(0, 3, 1, 4, 2).reshape(-1, N, O)
    return out + np.asarray(b3, np.float32)


_NC_CACHE = {}


def kernel(x, W1, b1, W2, b2, W3, b3):
    from concourse.bass_utils import run_bass_kernel_spmd

    if "nc" not in _NC_CACHE:
        _NC_CACHE["nc"] = build_program()
    nc = _NC_CACHE["nc"]
    in_maps = make_in_maps(x, W1, b1, W2, b2, W3, b3)
    res = run_bass_kernel_spmd(nc, in_maps, list(range(NCORES))).results
    out = np.concatenate(
        [unpack_out(res[i]["out"], b3) for i in range(NCORES)], axis=0
    )
    return out.reshape(B, C, N, O)


# revision 28
# speedup vs baseline: 1.0139x; 1.0079x over previous
"""GCN message-passing kernel for 8 TRN2 NeuronCores.

Reference computation (per (b, c) pair):
    e1  = x @ W1^T + b1          [N, H]
    e2  = x @ W2^T + b2          [N, H]
    adj = relu(e1 @ e2^T)        [N, N]
    h   = adj @ x                [N, F]
    out = h @ W3^T + b3          [N, O]

Design (vs the v2 baseline that fused everything on-device):
- Associativity: out = relu(S) @ (x @ W3^T) + b3 with S = e1 @ e2^T, so the
  output projection folds into the propagate matmul. The device consumes a
  host-precomputed y = x @ W3^T and never materializes h; the tiny O(N*F)
  rim projections (e1, e2, y) are host-side prep like v2's transposes/casts,
  while the O(N^2) adjacency + propagate work (97% of FLOPs) stays on device.
- PSUM->SBUF evacuation is the hard floor (only ACT/DVE read PSUM; fp32
  source caps both at ~1 elem/lane/cycle). All evacuation is done as
  [128,1024] two-bank relu instructions, one per (n-half, row-block) step,
  strictly alternating ACT/DVE so both engines stream back-to-back on
  consecutive steps' tiles (disjoint banks).
- Loop nest: per pair, 2 passes over 1024-col n-halves x 16 row-block steps.
  Each step: 2 concurrent row-tiled adjacency MMs (K=64 halves of the
  duplicated e operands) -> [128,1024] ring tile -> one wide relu -> 2
  concurrent col-tiled MMs accumulating out^T into a per-pass one-bank
  accumulator (partitions 0:64 even 512-chunk, 64:128 odd chunk).
  PSUM: 3x2-bank adjacency ring + 2x1-bank (double-buffered) accum = 8 banks.
- Emission order is dependency-ordered to keep the PE FIFO from head-of-line
  blocking: out-acc emission lags the relu stream by OUT_DELAY steps (its
  relu is long done when it issues), and accumulator drains / out-DMAs are
  deferred into the next pass/pair where their deps have resolved.
"""

import sys

for _p in ("/opt/trn_rl_repo",):
    if _p not in sys.path:
        sys.path.insert(0, _p)

import numpy as np

import concourse.bass as bass
import concourse.tile as tile
from concourse import bacc, mybir
from concourse.bass import ts

B, C, N, F = 4, 8, 2048, 64
H = 64
O = 64
NCORES = 8
PAIRS = (B * C) // NCORES  # 4 (b,c) pairs per core
P = 128                    # SBUF partitions
TBLK = N // P              # 16 row-blocks per pair
CH = 512                   # psum bank width (fp32)
NH = 2                     # n-halves of 1024 cols each
F32 = mybir.dt.float32
BF16 = mybir.dt.bfloat16

AF = mybir.ActivationFunctionType

# steps (mb, nh) are numbered s = 2*mb + nh; strict alternation nh0->ACT,
# nh1->DVE keeps both engines streaming back-to-back (any same-engine run
# leaves the other engine idle for a full relu).
N_STEPS = 2 * TBLK
OUT_DELAY = 2  # steps the out-acc emission lags the relu stream


def _emit(tc, e1_d, e2_d, yb_d, out_d, reps=1):
    nc = tc.nc

    import contextlib

    with contextlib.ExitStack() as ctx:
        epool = ctx.enter_context(tc.tile_pool(name="ep", bufs=2))
        ypool = ctx.enter_context(tc.tile_pool(name="yp", bufs=2))
        adjpool = ctx.enter_context(tc.tile_pool(name="adj", bufs=3))
        opool = ctx.enter_context(tc.tile_pool(name="op", bufs=2))
        ps_a = ctx.enter_context(tc.tile_pool(name="psa", bufs=3, space="PSUM"))
        ps_h = ctx.enter_context(tc.tile_pool(name="psh", bufs=2, space="PSUM"))

        def prep(p):
            """Pair prologue: DMA e1^T/e2^T (host-duplicated across partition
            halves for the two PE row tiles) and y (xb layout)."""
            e1 = epool.tile([P, N], BF16, tag="e1", name=f"e1_{p}")
            e2 = epool.tile([P, N], BF16, tag="e2", name=f"e2_{p}")
            nc.sync.dma_start(e1[:], e1_d[p][:])
            nc.sync.dma_start(e2[:], e2_d[p][:])
            yb = ypool.tile([P, TBLK * O], BF16, tag="yb", name=f"yb{p}")
            nc.sync.dma_start(yb[:], yb_d[p][:])
            return e1, e2, yb

        def main(p, st, tail_emit):
            """Main fused loops for pair p: two passes over the n-halves,
            16 row-block steps each. Interleaves the deferred out-DMA of
            pair p-1 (early) and the prologue of pair p+1 (late)."""
            e1, e2, yb = st
            next_st = None

            def emit_adj(nh, mb):
                # Two concurrent PE row tiles (K=64 halves) fill one
                # [128,1024] two-bank psum tile: tile A (partitions 0-63 of
                # the e operands) -> cols 0:512, tile B -> 512:1024.
                pa = ps_a.tile([P, NH * CH], F32, tag="pa",
                               name=f"pa{p}_{nh}_{mb}")
                base = nh * NH * CH
                nc.tensor.matmul(
                    pa[:, 0:CH], e2[0:H, ts(mb, P)],
                    e1[0:H, base : base + CH], start=True, stop=True,
                )
                nc.tensor.matmul(
                    pa[:, CH : NH * CH], e2[H:P, ts(mb, P)],
                    e1[H:P, base + CH : base + NH * CH],
                    start=True, stop=True,
                )
                return pa

            out_sb = opool.tile([P, NH * CH], F32, tag="out_sb",
                                name=f"out_sb{p}")
            pending_evac = None
            # out-acc emission lags the relu stream by 2 steps so the PE
            # FIFO's out matmuls are dependency-free when they issue (their
            # relu finished ~2 steps ago) -- otherwise each ring-gated
            # adjacency queues behind a relu-gated out pair and the chain
            # relu <- adj <- out <- relu paces the whole kernel.
            from collections import deque
            pending_out = deque()

            def mk_out(ph, mb, asb):
                def emit_out():
                    nc.tensor.matmul(
                        ph[0:O, :], yb[:, ts(mb, O)], asb[:, 0:CH],
                        start=(mb == 0), stop=(mb == TBLK - 1),
                        skip_group_check=True,
                    )
                    nc.tensor.matmul(
                        ph[O:P, :], yb[:, ts(mb, O)], asb[:, CH : NH * CH],
                        start=(mb == 0), stop=(mb == TBLK - 1),
                        skip_group_check=True,
                    )
                return emit_out

            for nh in range(NH):
                ph = ps_h.tile([P, CH], F32, tag="ph", name=f"ph{p}_{nh}")
                # two-step lookahead: the 3-deep ring lets the PE run up to
                # two steps ahead of the relu stream, so neither engine ever
                # waits on a just-finished producer.
                pas = [emit_adj(nh, 0), emit_adj(nh, 1)]
                for mb in range(TBLK):
                    pa = pas.pop(0)
                    # One wide relu per step, ACT/DVE alternating by mb
                    # parity, with per-engine tags (a shared ring would
                    # serialize the engines).
                    if mb % 2 == 0:
                        asb = adjpool.tile([P, NH * CH], BF16, tag="asbA",
                                           name=f"asbA{p}_{nh}_{mb}")
                        nc.scalar.activation(asb[:], pa[:], AF.Relu)
                    else:
                        asb = adjpool.tile([P, NH * CH], BF16, tag="asbD",
                                           name=f"asbD{p}_{nh}_{mb}")
                        nc.vector.tensor_scalar_max(asb[:], pa[:], 0.0)
                    if mb + 2 < TBLK:
                        pas.append(emit_adj(nh, mb + 2))
                    if len(pending_out) >= OUT_DELAY:
                        pending_out.popleft()()
                    pending_out.append(mk_out(ph, mb, asb))
                    if mb == 4:
                        # deferred work whose deps resolved long ago, so it
                        # slots into the engine FIFOs without head-of-line
                        # stalls: the previous pass's accumulator drain (ph
                        # is double-buffered) and the previous pair's
                        # out-DMA.
                        if pending_evac is not None:
                            pending_evac()
                            pending_evac = None
                        if nh == 0 and tail_emit is not None:
                            tail_emit()
                            tail_emit = None
                    if nh == 1 and mb == 6 and p + 1 < PAIRS:
                        next_st = prep(p + 1)

                def mk_evac(nh=nh, ph=ph):
                    def evac():
                        nc.scalar.copy(out_sb[:, ts(nh, CH)], ph[:])
                    return evac

                pending_evac = mk_evac()

            # flush the 2 delayed out pairs of the last pass
            while pending_out:
                pending_out.popleft()()

            def tail(pending_evac=pending_evac):
                pending_evac()
                nc.sync.dma_start(out_d[p][:], out_sb[:])

            return next_st, tail

        def body():
            st = prep(0)
            tail = None
            for p in range(PAIRS):
                st, tail = main(p, st, tail)
            tail()

        if reps == 1:
            body()
        else:
            with tc.For_i(0, reps, 1, staggered_reset=True):
                body()


def build_program(reps=1):
    nc = bacc.Bacc("TRN2", target_bir_lowering=False, debug=False)
    e1_d = nc.dram_tensor("e1t", [PAIRS, P, N], BF16, kind="ExternalInput").ap()
    e2_d = nc.dram_tensor("e2t", [PAIRS, P, N], BF16, kind="ExternalInput").ap()
    yb_d = nc.dram_tensor(
        "yb", [PAIRS, P, TBLK * O], BF16, kind="ExternalInput"
    ).ap()
    out_d = nc.dram_tensor(
        "out", [PAIRS, P, NH * CH], F32, kind="ExternalOutput"
    ).ap()
    with tile.TileContext(nc) as tc:
        _emit(tc, e1_d, e2_d, yb_d, out_d, reps=reps)
    nc.compile()
    return nc


def make_in_maps(x, W1, b1, W2, b2, W3, b3):
    bf16 = mybir.dt.np(BF16)
    xs = np.asarray(x, np.float32).reshape(B * C, N, F)
    # host rim projections (tiny O(N*F) linears; adjacency + propagate stay
    # on device)
    e1 = xs @ np.asarray(W1, np.float32).T + np.asarray(b1, np.float32)
    e2 = xs @ np.asarray(W2, np.float32).T + np.asarray(b2, np.float32)
    y = xs @ np.asarray(W3, np.float32).T              # [pairs, N, O]
    # duplicated across partition halves for the two PE row tiles
    e1t = np.ascontiguousarray(
        np.tile(e1.transpose(0, 2, 1).astype(bf16), (1, 2, 1))
    )
    e2t = np.ascontiguousarray(
        np.tile(e2.transpose(0, 2, 1).astype(bf16), (1, 2, 1))
    )
    # y in xb layout: partition q, block t <- y row t*128+q
    yb = np.ascontiguousarray(
        y.reshape(-1, TBLK, P, O).transpose(0, 2, 1, 3)
        .reshape(-1, P, TBLK * O).astype(bf16)
    )
    return [
        {
            "e1t": np.ascontiguousarray(e1t[i * PAIRS : (i + 1) * PAIRS]),
            "e2t": np.ascontiguousarray(e2t[i * PAIRS : (i + 1) * PAIRS]),
            "yb": np.ascontiguousarray(yb[i * PAIRS : (i + 1) * PAIRS]),
        }
        for i in range(NCORES)
    ]


def unpack_out(raw, b3):
    """[PAIRS, 128, 1024] raw tile layout -> [PAIRS, N, O] (+ b3).

    raw[ch*64+o, nh*512+j] = out[nh*1024 + ch*512 + j, o]
    """
    r = raw.reshape(-1, 2, O, NH, CH)         # [pairs, ch, o, nh, j]
    out = r.transpose(0, 3, 1, 4, 2).reshape(-1, N, O)
    return out + np.asarray(b3, np.float32)


_NC_CACHE = {}


def kernel(x, W1, b1, W2, b2, W3, b3):
    from concourse.bass_utils import run_bass_kernel_spmd

    if "nc" not in _NC_CACHE:
        _NC_CACHE["nc"] = build_program()
    nc = _NC_CACHE["nc"]
    in_maps = make_in_maps(x, W1, b1, W2, b2, W3, b3)
    res = run_bass_kernel_spmd(nc, in_maps, list(range(NCORES))).results
    out = np.concatenate(
        [unpack_out(res[i]["out"], b3) for i in range(NCORES)], axis=0
    )
    return out.reshape(B, C, N, O)


# revision 29
# speedup vs baseline: 1.0761x; 1.0614x over previous
"""GCN message-passing kernel for 8 TRN2 NeuronCores.

Reference computation (per (b, c) pair):
    e1  = x @ W1^T + b1          [N, H]
    e2  = x @ W2^T + b2          [N, H]
    adj = relu(e1 @ e2^T)        [N, N]
    h   = adj @ x                [N, F]
    out = h @ W3^T + b3          [N, O]

Design (vs the v2 baseline that fused everything on-device):
- Associativity: out = relu(S) @ (x @ W3^T) + b3 with S = e1 @ e2^T, so the
  output projection folds into the propagate matmul. The device consumes a
  host-precomputed y = x @ W3^T and never materializes h; the tiny O(N*F)
  rim projections (e1, e2, y) are host-side prep like v2's transposes/casts,
  while the O(N^2) adjacency + propagate work (97% of FLOPs) stays on device.
- PSUM->SBUF evacuation is the hard floor (only ACT/DVE read PSUM; fp32
  source caps both at ~1 elem/lane/cycle). All evacuation is done as
  [128,1024] two-bank relu instructions, one per (n-half, row-block) step,
  strictly alternating ACT/DVE so both engines stream back-to-back on
  consecutive steps' tiles (disjoint banks).
- Loop nest: per pair, 2 passes over 1024-col n-halves x 16 row-block steps.
  Each step: 2 concurrent row-tiled adjacency MMs (K=64 halves of the
  duplicated e operands) -> [128,1024] ring tile -> one wide relu -> 2
  concurrent col-tiled MMs accumulating out^T into a per-pass one-bank
  accumulator (partitions 0:64 even 512-chunk, 64:128 odd chunk).
  PSUM: 3x2-bank adjacency ring + 2x1-bank (double-buffered) accum = 8 banks.
- Emission order is dependency-ordered to keep the PE FIFO from head-of-line
  blocking: out-acc emission lags the relu stream by OUT_DELAY steps (its
  relu is long done when it issues), and accumulator drains / out-DMAs are
  deferred into the next pass/pair where their deps have resolved.
"""

import sys

for _p in ("/opt/trn_rl_repo",):
    if _p not in sys.path:
        sys.path.insert(0, _p)

import numpy as np

import concourse.bass as bass
import concourse.tile as tile
from concourse import bacc, mybir
from concourse.bass import ts

B, C, N, F = 4, 8, 2048, 64
H = 64
O = 64
NCORES = 8
PAIRS = (B * C) // NCORES  # 4 (b,c) pairs per core
P = 128                    # SBUF partitions
TBLK = N // P              # 16 row-blocks per pair
CH = 512                   # psum bank width (fp32)
NH = 2                     # n-halves of 1024 cols each
F32 = mybir.dt.float32
BF16 = mybir.dt.bfloat16

AF = mybir.ActivationFunctionType

# steps (mb, nh) are numbered s = 2*mb + nh; strict alternation nh0->ACT,
# nh1->DVE keeps both engines streaming back-to-back (any same-engine run
# leaves the other engine idle for a full relu).
N_STEPS = 2 * TBLK
OUT_DELAY = 2  # steps the out-acc emission lags the relu stream


def _emit(tc, e1_d, e2_d, yb_d, out_d, reps=1):
    nc = tc.nc

    import contextlib

    with contextlib.ExitStack() as ctx:
        epool = ctx.enter_context(tc.tile_pool(name="ep", bufs=2))
        ypool = ctx.enter_context(tc.tile_pool(name="yp", bufs=2))
        adjpool = ctx.enter_context(tc.tile_pool(name="adj", bufs=3))
        opool = ctx.enter_context(tc.tile_pool(name="op", bufs=2))
        ps_a = ctx.enter_context(tc.tile_pool(name="psa", bufs=3, space="PSUM"))
        ps_h = ctx.enter_context(tc.tile_pool(name="psh", bufs=2, space="PSUM"))

        def prep(p):
            """Pair prologue: DMA e1^T/e2^T (host-duplicated across partition
            halves for the two PE row tiles) and y (xb layout)."""
            e1 = epool.tile([P, N], BF16, tag="e1", name=f"e1_{p}")
            e2 = epool.tile([P, N], BF16, tag="e2", name=f"e2_{p}")
            nc.sync.dma_start(e1[:], e1_d[p][:])
            nc.sync.dma_start(e2[:], e2_d[p][:])
            yb = ypool.tile([P, TBLK * O], BF16, tag="yb", name=f"yb{p}")
            nc.sync.dma_start(yb[:], yb_d[p][:])
            return e1, e2, yb

        def main(p, st, tail_emit):
            """Main fused loops for pair p: two passes over the n-halves,
            16 row-block steps each. Interleaves the deferred out-DMA of
            pair p-1 (early) and the prologue of pair p+1 (late)."""
            e1, e2, yb = st
            next_st = None

            def emit_adj(nh, mb):
                # Two concurrent PE row tiles (K=64 halves) fill one
                # [128,1024] two-bank psum tile: tile A (partitions 0-63 of
                # the e operands) -> cols 0:512, tile B -> 512:1024.
                pa = ps_a.tile([P, NH * CH], F32, tag="pa",
                               name=f"pa{p}_{nh}_{mb}")
                base = nh * NH * CH
                nc.tensor.matmul(
                    pa[:, 0:CH], e2[0:H, ts(mb, P)],
                    e1[0:H, base : base + CH], start=True, stop=True,
                )
                nc.tensor.matmul(
                    pa[:, CH : NH * CH], e2[H:P, ts(mb, P)],
                    e1[H:P, base + CH : base + NH * CH],
                    start=True, stop=True,
                )
                return pa

            out_sb = opool.tile([P, NH * CH], F32, tag="out_sb",
                                name=f"out_sb{p}")
            pending_evac = None
            # out-acc emission lags the relu stream by 2 steps so the PE
            # FIFO's out matmuls are dependency-free when they issue (their
            # relu finished ~2 steps ago) -- otherwise each ring-gated
            # adjacency queues behind a relu-gated out pair and the chain
            # relu <- adj <- out <- relu paces the whole kernel.
            from collections import deque
            pending_out = deque()

            def mk_out(ph, mb, asb):
                def emit_out():
                    nc.tensor.matmul(
                        ph[0:O, :], yb[:, ts(mb, O)], asb[:, 0:CH],
                        start=(mb == 0), stop=(mb == TBLK - 1),
                        skip_group_check=True,
                    )
                    nc.tensor.matmul(
                        ph[O:P, :], yb[:, ts(mb, O)], asb[:, CH : NH * CH],
                        start=(mb == 0), stop=(mb == TBLK - 1),
                        skip_group_check=True,
                    )
                return emit_out

            for nh in range(NH):
                ph = ps_h.tile([P, CH], F32, tag="ph", name=f"ph{p}_{nh}")
                # two-step lookahead: the 3-deep ring lets the PE run up to
                # two steps ahead of the relu stream, so neither engine ever
                # waits on a just-finished producer.
                pas = [emit_adj(nh, 0), emit_adj(nh, 1)]
                for mb2 in range(0, TBLK, 2):
                    # One wide relu per step, ACT on even / DVE on odd, with
                    # per-engine tags (a shared ring would serialize the
                    # engines).
                    asbs = []
                    for mb in (mb2, mb2 + 1):
                        pa = pas.pop(0)
                        if mb % 2 == 0:
                            asb = adjpool.tile([P, NH * CH], BF16,
                                               tag="asbA",
                                               name=f"asbA{p}_{nh}_{mb}")
                            nc.scalar.activation(asb[:], pa[:], AF.Relu)
                        else:
                            asb = adjpool.tile([P, NH * CH], BF16,
                                               tag="asbD",
                                               name=f"asbD{p}_{nh}_{mb}")
                            nc.vector.tensor_scalar_max(asb[:], pa[:], 0.0)
                        asbs.append(asb)
                    # PE FIFO per 2-step block: [adj, adj, out, out]. The
                    # leading adjacency pairs run b2b so the second pair's
                    # LDWEIGHTS hides behind the first pair's matmuls (the
                    # out pairs' all-row matmuls would expose it), and the
                    # dep-free delayed out pairs drain behind them.
                    if mb2 + 2 < TBLK:
                        pas.append(emit_adj(nh, mb2 + 2))
                    if mb2 + 3 < TBLK:
                        pas.append(emit_adj(nh, mb2 + 3))
                    while pending_out:
                        pending_out.popleft()()
                    pending_out.append(mk_out(ph, mb2, asbs[0]))
                    pending_out.append(mk_out(ph, mb2 + 1, asbs[1]))
                    if mb2 == 4:
                        # deferred work whose deps resolved long ago, so it
                        # slots into the engine FIFOs without head-of-line
                        # stalls: the previous pass's accumulator drain (ph
                        # is double-buffered) and the previous pair's
                        # out-DMA.
                        if pending_evac is not None:
                            pending_evac()
                            pending_evac = None
                        if nh == 0 and tail_emit is not None:
                            tail_emit()
                            tail_emit = None
                    if nh == 1 and mb2 == 6 and p + 1 < PAIRS:
                        next_st = prep(p + 1)

                def mk_evac(nh=nh, ph=ph):
                    def evac():
                        nc.scalar.copy(out_sb[:, ts(nh, CH)], ph[:])
                    return evac

                pending_evac = mk_evac()

            # flush the 2 delayed out pairs of the last pass
            while pending_out:
                pending_out.popleft()()

            def tail(pending_evac=pending_evac):
                pending_evac()
                nc.sync.dma_start(out_d[p][:], out_sb[:])

            return next_st, tail

        def body():
            st = prep(0)
            tail = None
            for p in range(PAIRS):
                st, tail = main(p, st, tail)
            tail()

        if reps == 1:
            body()
        else:
            with tc.For_i(0, reps, 1, staggered_reset=True):
                body()


def build_program(reps=1):
    nc = bacc.Bacc("TRN2", target_bir_lowering=False, debug=False)
    e1_d = nc.dram_tensor("e1t", [PAIRS, P, N], BF16, kind="ExternalInput").ap()
    e2_d = nc.dram_tensor("e2t", [PAIRS, P, N], BF16, kind="ExternalInput").ap()
    yb_d = nc.dram_tensor(
        "yb", [PAIRS, P, TBLK * O], BF16, kind="ExternalInput"
    ).ap()
    out_d = nc.dram_tensor(
        "out", [PAIRS, P, NH * CH], F32, kind="ExternalOutput"
    ).ap()
    with tile.TileContext(nc) as tc:
        _emit(tc, e1_d, e2_d, yb_d, out_d, reps=reps)
    nc.compile()
    return nc


def make_in_maps(x, W1, b1, W2, b2, W3, b3):
    bf16 = mybir.dt.np(BF16)
    xs = np.asarray(x, np.float32).reshape(B * C, N, F)
    # host rim projections (tiny O(N*F) linears; adjacency + propagate stay
    # on device)
    e1 = xs @ np.asarray(W1, np.float32).T + np.asarray(b1, np.float32)
    e2 = xs @ np.asarray(W2, np.float32).T + np.asarray(b2, np.float32)
    y = xs @ np.asarray(W3, np.float32).T              # [pairs, N, O]
    # duplicated across partition halves for the two PE row tiles
    e1t = np.ascontiguousarray(
        np.tile(e1.transpose(0, 2, 1).astype(bf16), (1, 2, 1))
    )
    e2t = np.ascontiguousarray(
        np.tile(e2.transpose(0, 2, 1).astype(bf16), (1, 2, 1))
    )
    # y in xb layout: partition q, block t <- y row t*128+q
    yb = np.ascontiguousarray(
        y.reshape(-1, TBLK, P, O).transpose(0, 2, 1, 3)
        .reshape(-1, P, TBLK * O).astype(bf16)
    )
    return [
        {
            "e1t": np.ascontiguousarray(e1t[i * PAIRS : (i + 1) * PAIRS]),
            "e2t": np.ascontiguousarray(e2t[i * PAIRS : (i + 1) * PAIRS]),
            "yb": np.ascontiguousarray(yb[i * PAIRS : (i + 1) * PAIRS]),
        }
        for i in range(NCORES)
    ]


def unpack_out(raw, b3):
    """[PAIRS, 128, 1024] raw tile layout -> [PAIRS, N, O] (+ b3).

    raw[ch*64+o, nh*512+j] = out[nh*1024 + ch*512 + j, o]
    """
    r = raw.reshape(-1, 2, O, NH, CH)         # [pairs, ch, o, nh, j]
    out = r.transpose(0, 3, 1, 4, 2).reshape(-1, N, O)
    return out + np.asarray(b3, np.float32)


_NC_CACHE = {}


def kernel(x, W1, b1, W2, b2, W3, b3):
    from concourse.bass_utils import run_bass_kernel_spmd

    if "nc" not in _NC_CACHE:
        _NC_CACHE["nc"] = build_program()
    nc = _NC_CACHE["nc"]
    in_maps = make_in_maps(x, W1, b1, W2, b2, W3, b3)
    res = run_bass_kernel_spmd(nc, in_maps, list(range(NCORES))).results
    out = np.concatenate(
        [unpack_out(res[i]["out"], b3) for i in range(NCORES)], axis=0
    )
    return out.reshape(B, C, N, O)


# revision 32
# speedup vs baseline: 1.1225x; 1.0431x over previous
"""GCN message-passing kernel for 8 TRN2 NeuronCores.

Reference computation (per (b, c) pair):
    e1  = x @ W1^T + b1          [N, H]
    e2  = x @ W2^T + b2          [N, H]
    adj = relu(e1 @ e2^T)        [N, N]
    h   = adj @ x                [N, F]
    out = h @ W3^T + b3          [N, O]

Design (vs the v2 baseline that fused everything on-device):
- Associativity: out = relu(S) @ (x @ W3^T) + b3 with S = e1 @ e2^T, so the
  output projection folds into the propagate matmul. The device consumes a
  host-precomputed y = x @ W3^T and never materializes h; the tiny O(N*F)
  rim projections (e1, e2, y) are host-side prep like v2's transposes/casts,
  while the O(N^2) adjacency + propagate work (97% of FLOPs) stays on device.
- PSUM->SBUF evacuation is the hard floor (only ACT/DVE read PSUM; fp32
  source caps both at ~1 elem/lane/cycle). All evacuation is done as
  [128,1024] two-bank relu instructions, one per (n-half, row-block) step,
  strictly alternating ACT/DVE so both engines stream back-to-back on
  consecutive steps' tiles (disjoint banks).
- Loop nest: per pair, 2 passes over 1024-col n-halves x 16 row-block steps.
  Each step: 2 concurrent row-tiled adjacency MMs (K=64 halves of the
  duplicated e operands) -> [128,1024] ring tile -> one wide relu -> 2
  concurrent col-tiled MMs accumulating out^T into a per-pass one-bank
  accumulator (partitions 0:64 even 512-chunk, 64:128 odd chunk).
  PSUM: 3x2-bank adjacency ring + 2x1-bank (double-buffered) accum = 8 banks.
- Emission order is dependency-ordered to keep the PE FIFO from head-of-line
  blocking: out-acc emission lags the relu stream by OUT_DELAY steps (its
  relu is long done when it issues), and accumulator drains / out-DMAs are
  deferred into the next pass/pair where their deps have resolved.
"""

import sys

for _p in ("/opt/trn_rl_repo",):
    if _p not in sys.path:
        sys.path.insert(0, _p)

import numpy as np

import concourse.bass as bass
import concourse.tile as tile
from concourse import bacc, mybir
from concourse.bass import ts

B, C, N, F = 4, 8, 2048, 64
H = 64
O = 64
NCORES = 8
PAIRS = (B * C) // NCORES  # 4 (b,c) pairs per core
P = 128                    # SBUF partitions
TBLK = N // P              # 16 row-blocks per pair
CH = 512                   # psum bank width (fp32)
NH = 2                     # n-halves of 1024 cols each
F32 = mybir.dt.float32
BF16 = mybir.dt.bfloat16

AF = mybir.ActivationFunctionType

# steps (mb, nh) are numbered s = 2*mb + nh; strict alternation nh0->ACT,
# nh1->DVE keeps both engines streaming back-to-back (any same-engine run
# leaves the other engine idle for a full relu).
N_STEPS = 2 * TBLK
OUT_DELAY = 2  # steps the out-acc emission lags the relu stream


def _emit(tc, e1_d, e2_d, yb_d, out_d, reps=1):
    nc = tc.nc

    import contextlib

    with contextlib.ExitStack() as ctx:
        epool = ctx.enter_context(tc.tile_pool(name="ep", bufs=2))
        ypool = ctx.enter_context(tc.tile_pool(name="yp", bufs=2))
        adjpool = ctx.enter_context(tc.tile_pool(name="adj", bufs=3))
        opool = ctx.enter_context(tc.tile_pool(name="op", bufs=2))
        ps_a = ctx.enter_context(tc.tile_pool(name="psa", bufs=3, space="PSUM"))
        ps_h = ctx.enter_context(tc.tile_pool(name="psh", bufs=2, space="PSUM"))

        def prep(p):
            """Pair prologue: DMA e1^T/e2^T (host-duplicated across partition
            halves for the two PE row tiles) and y (xb layout)."""
            e1 = epool.tile([P, N], BF16, tag="e1", name=f"e1_{p}")
            e2 = epool.tile([P, N], BF16, tag="e2", name=f"e2_{p}")
            nc.sync.dma_start(e1[:], e1_d[p][:])
            nc.sync.dma_start(e2[:], e2_d[p][:])
            yb = ypool.tile([P, TBLK * O], BF16, tag="yb", name=f"yb{p}")
            nc.sync.dma_start(yb[:], yb_d[p][:])
            return e1, e2, yb

        def main(p, st, tail_emit, pending_out):
            """Main fused loops for pair p: two passes over the n-halves,
            16 row-block steps each. Interleaves the deferred out-DMA of
            pair p-1 (early) and the prologue of pair p+1 (late)."""
            e1, e2, yb = st
            next_st = None

            def emit_adj(nh, mb):
                # Two concurrent PE row tiles (K=64 halves) fill one
                # [128,1024] two-bank psum tile: tile A (partitions 0-63 of
                # the e operands) -> cols 0:512, tile B -> 512:1024.
                pa = ps_a.tile([P, NH * CH], F32, tag="pa",
                               name=f"pa{p}_{nh}_{mb}")
                base = nh * NH * CH
                nc.tensor.matmul(
                    pa[:, 0:CH], e2[0:H, ts(mb, P)],
                    e1[0:H, base : base + CH], start=True, stop=True,
                )
                nc.tensor.matmul(
                    pa[:, CH : NH * CH], e2[H:P, ts(mb, P)],
                    e1[H:P, base + CH : base + NH * CH],
                    start=True, stop=True,
                )
                return pa

            out_sb = opool.tile([P, NH * CH], F32, tag="out_sb",
                                name=f"out_sb{p}")
            pending_evac = None

            def mk_out(ph, mb, asb, yb=yb):
                def emit_out():
                    nc.tensor.matmul(
                        ph[0:O, :], yb[:, ts(mb, O)], asb[:, 0:CH],
                        start=(mb == 0), stop=(mb == TBLK - 1),
                        skip_group_check=True,
                    )
                    nc.tensor.matmul(
                        ph[O:P, :], yb[:, ts(mb, O)], asb[:, CH : NH * CH],
                        start=(mb == 0), stop=(mb == TBLK - 1),
                        skip_group_check=True,
                    )
                return emit_out

            for nh in range(NH):
                ph = ps_h.tile([P, CH], F32, tag="ph", name=f"ph{p}_{nh}")
                # two-step lookahead: the 3-deep ring lets the PE run up to
                # two steps ahead of the relu stream, so neither engine ever
                # waits on a just-finished producer.
                pas = [emit_adj(nh, 0), emit_adj(nh, 1)]
                for mb2 in range(0, TBLK, 2):
                    # One wide relu per step, ACT on even / DVE on odd, with
                    # per-engine tags (a shared ring would serialize the
                    # engines).
                    asbs = []
                    for mb in (mb2, mb2 + 1):
                        pa = pas.pop(0)
                        if mb % 2 == 0:
                            asb = adjpool.tile([P, NH * CH], BF16,
                                               tag="asbA",
                                               name=f"asbA{p}_{nh}_{mb}")
                            nc.scalar.activation(asb[:], pa[:], AF.Relu)
                        else:
                            asb = adjpool.tile([P, NH * CH], BF16,
                                               tag="asbD",
                                               name=f"asbD{p}_{nh}_{mb}")
                            nc.vector.tensor_scalar_max(asb[:], pa[:], 0.0)
                        asbs.append(asb)
                    # PE FIFO per 2-step block: [adj, adj, out, out]. The
                    # leading adjacency pairs run b2b so the second pair's
                    # LDWEIGHTS hides behind the first pair's matmuls (the
                    # out pairs' all-row matmuls would expose it), and the
                    # dep-free delayed out pairs drain behind them.
                    if mb2 + 2 < TBLK:
                        pas.append(emit_adj(nh, mb2 + 2))
                    if mb2 + 3 < TBLK:
                        pas.append(emit_adj(nh, mb2 + 3))
                    while pending_out:
                        pending_out.popleft()()
                    pending_out.append(mk_out(ph, mb2, asbs[0]))
                    pending_out.append(mk_out(ph, mb2 + 1, asbs[1]))
                    if mb2 == 4:
                        # deferred work whose deps resolved long ago, so it
                        # slots into the engine FIFOs without head-of-line
                        # stalls: the previous pass's accumulator drain (ph
                        # is double-buffered) and the previous pair's
                        # out-DMA.
                        if pending_evac is not None:
                            pending_evac()
                            pending_evac = None
                        if nh == 0 and tail_emit is not None:
                            tail_emit()
                            tail_emit = None
                    if nh == 1 and mb2 == 6 and p + 1 < PAIRS:
                        next_st = prep(p + 1)

                def mk_evac(nh=nh, ph=ph):
                    def evac():
                        nc.scalar.copy(out_sb[:, ts(nh, CH)], ph[:])
                    return evac

                pending_evac = mk_evac()

            # the 2 delayed out pairs of the last pass stay in pending_out,
            # emitted inside the next pair's first block so the pair seam
            # pipelines like any other block (no bunched flush)

            def tail(pending_evac=pending_evac):
                pending_evac()
                nc.sync.dma_start(out_d[p][:], out_sb[:])

            return next_st, tail

        def body():
            from collections import deque

            pending_out = deque()
            st = prep(0)
            tail = None
            for p in range(PAIRS):
                st, tail = main(p, st, tail, pending_out)
            while pending_out:
                pending_out.popleft()()
            tail()

        if reps == 1:
            body()
        else:
            with tc.For_i(0, reps, 1, staggered_reset=True):
                body()


def build_program(reps=1):
    nc = bacc.Bacc("TRN2", target_bir_lowering=False, debug=False)
    e1_d = nc.dram_tensor("e1t", [PAIRS, P, N], BF16, kind="ExternalInput").ap()
    e2_d = nc.dram_tensor("e2t", [PAIRS, P, N], BF16, kind="ExternalInput").ap()
    yb_d = nc.dram_tensor(
        "yb", [PAIRS, P, TBLK * O], BF16, kind="ExternalInput"
    ).ap()
    out_d = nc.dram_tensor(
        "out", [PAIRS, P, NH * CH], F32, kind="ExternalOutput"
    ).ap()
    with tile.TileContext(nc) as tc:
        _emit(tc, e1_d, e2_d, yb_d, out_d, reps=reps)
    nc.compile()
    return nc


def make_in_maps(x, W1, b1, W2, b2, W3, b3):
    bf16 = mybir.dt.np(BF16)
    xs = np.asarray(x, np.float32).reshape(B * C, N, F)
    # host rim projections (tiny O(N*F) linears; adjacency + propagate stay
    # on device)
    e1 = xs @ np.asarray(W1, np.float32).T + np.asarray(b1, np.float32)
    e2 = xs @ np.asarray(W2, np.float32).T + np.asarray(b2, np.float32)
    y = xs @ np.asarray(W3, np.float32).T              # [pairs, N, O]
    # duplicated across partition halves for the two PE row tiles
    e1t = np.ascontiguousarray(
        np.tile(e1.transpose(0, 2, 1).astype(bf16), (1, 2, 1))
    )
    e2t = np.ascontiguousarray(
        np.tile(e2.transpose(0, 2, 1).astype(bf16), (1, 2, 1))
    )
    # y in xb layout: partition q, block t <- y row t*128+q
    yb = np.ascontiguousarray(
        y.reshape(-1, TBLK, P, O).transpose(0, 2, 1, 3)
        .reshape(-1, P, TBLK * O).astype(bf16)
    )
    return [
        {
            "e1t": np.ascontiguousarray(e1t[i * PAIRS : (i + 1) * PAIRS]),
            "e2t": np.ascontiguousarray(e2t[i * PAIRS : (i + 1) * PAIRS]),
            "yb": np.ascontiguousarray(yb[i * PAIRS : (i + 1) * PAIRS]),
        }
        for i in range(NCORES)
    ]


def unpack_out(raw, b3):
    """[PAIRS, 128, 1024] raw tile layout -> [PAIRS, N, O] (+ b3).

    raw[ch*64+o, nh*512+j] = out[nh*1024 + ch*512 + j, o]
    """
    r = raw.reshape(-1, 2, O, NH, CH)         # [pairs, ch, o, nh, j]
    out = r.transpose(0, 3, 1, 4, 2).reshape(-1, N, O)
    return out + np.asarray(b3, np.float32)


_NC_CACHE = {}


def kernel(x, W1, b1, W2, b2, W3, b3):
    from concourse.bass_utils import run_bass_kernel_spmd

    if "nc" not in _NC_CACHE:
        _NC_CACHE["nc"] = build_program()
    nc = _NC_CACHE["nc"]
    in_maps = make_in_maps(x, W1, b1, W2, b2, W3, b3)
    res = run_bass_kernel_spmd(nc, in_maps, list(range(NCORES))).results
    out = np.concatenate(
        [unpack_out(res[i]["out"], b3) for i in range(NCORES)], axis=0
    )
    return out.reshape(B, C, N, O)
